# revision 5
# baseline (speedup 1.0000x reference)
"""FP8 dynamic-quantized linear (x @ W + b with abs-max fp8 quantization).

Strategy (8 NeuronCores):
  - Shard: 2-way on flattened batch*seq rows of inp, 4-way column-wise on
    weight out_features.  Each core computes a [4096, 4096] block of the
    [8192, 16384] output (K = 4096 contraction on-device).
  - The two scalar quantization scales (global abs-max of inp / weight) are
    computed on host and replicated to every core as tiny input tensors.
  - Everything else (fp8 quantization of x and W, fp8 DoubleRow matmul,
    dequant scale + bias epilogue) runs on-device.

fp8 format note: TRN float8e4 (= ml_dtypes.float8_e4m3, max 240, has inf)
differs from the reference's OCP float8_e4m3fn (max 448).  We quantize with
half the reference scale so post-scale values live in [-224, 224]; on the
power-of-2-relative e4m3 grid the RNE rounding then matches the reference's
e4m3fn rounding exactly (up to a negligible subnormal tail), and the factor
of 4 (2x per operand) is folded into the fp32 dequant scale.
"""

import numpy as np

F8_MAX = np.float32(448.0)

# ---- problem geometry (hardcoded per the task spec) ----
B, T, K, OUTF = 4, 2048, 4096, 16384
ROWS = B * T                     # 8192
N_CORES = 8
ROW_SHARDS, COL_SHARDS = 2, 4
ROWS_C = ROWS // ROW_SHARDS      # 4096 rows per core
OUTF_C = OUTF // COL_SHARDS      # 4096 out-features per core

P = 128                          # SBUF partitions
KO = K // P                      # 32 k-subtiles
RT = ROWS_C // P                 # 32 row tiles per core
OC = 512                         # out-feature chunk (psum free dim)
NCHUNK = OUTF_C // OC            # 8 chunks per core
NPASS = 2                        # chunk passes (4 chunks resident per pass)
CPP = NCHUNK // NPASS            # chunks per pass = 4
KH = 8                           # ko-slices per w staging DMA


def _build_nc(rt=RT, ko=KO, nchunk=NCHUNK, npass=NPASS, oc=OC):
    """Build the per-core SPMD bass program (same program on all 8 cores)."""
    import concourse.bass as bass
    import concourse.tile as tile
    from concourse import bacc, mybir

    cpp = nchunk // npass
    outf_c = nchunk * oc
    f32 = mybir.dt.float32
    f8 = mybir.dt.float8e4
    DR = mybir.MatmulPerfMode.DoubleRow

    nc = bacc.Bacc(
        "TRN2",
        target_bir_lowering=False,
        debug=False,
        enable_asserts=False,
        num_devices=N_CORES,
    )

    xt = nc.dram_tensor("xt", [rt, P, ko, P], f32, kind="ExternalInput").ap()
    wt = nc.dram_tensor("wt", [nchunk, P, ko, oc], f32, kind="ExternalInput").ap()
    biasb = nc.dram_tensor("biasb", [P, outf_c], f32, kind="ExternalInput").ap()
    consts = nc.dram_tensor("consts", [P, 4], f32, kind="ExternalInput").ap()
    out = nc.dram_tensor("out", [rt, P, outf_c], f32, kind="ExternalOutput").ap()

    kh = min(KH, ko)

    with tile.TileContext(nc) as tc:
        with (
            tc.tile_pool(name="const", bufs=1) as const_pool,
            tc.tile_pool(name="wq", bufs=min(nchunk, cpp + 1)) as wq_pool,
            tc.tile_pool(name="xq", bufs=3) as xq_pool,
            tc.tile_pool(name="xf", bufs=2) as xf_pool,
            tc.tile_pool(name="wf", bufs=2) as wf_pool,
            tc.tile_pool(name="osb", bufs=2) as out_pool,
            tc.tile_pool(name="psum", bufs=8, space="PSUM") as psum_pool,
        ):
            consts_t = const_pool.tile([P, 4], f32)
            nc.sync.dma_start(consts_t[:], consts)
            rx_half = consts_t[:, 0:1]
            rw_half = consts_t[:, 1:2]
            c4 = consts_t[:, 2:3]

            biasb_t = const_pool.tile([P, outf_c], f32)
            nc.sync.dma_start(biasb_t[:], biasb)

            def load_wq_chunk(c):
                wq_c = wq_pool.tile([P, ko, oc], f8, tag="wq")
                for h in range(ko // kh):
                    wf = wf_pool.tile([P, kh, oc], f32, tag="wf")
                    nc.sync.dma_start(wf[:], wt[c, :, h * kh:(h + 1) * kh, :])
                    # w_q = fp8(w * (recip_w / 2)) on the scalar engine
                    nc.scalar.mul(wq_c[:, h * kh:(h + 1) * kh, :], wf[:], rw_half)
                return wq_c

            for p in range(npass):
                wq_chunks = [load_wq_chunk(p * cpp + i) for i in range(cpp)]
                for r in range(rt):
                    xf = xf_pool.tile([P, ko, P], f32, tag="xf")
                    nc.sync.dma_start(xf[:], xt[r])
                    xq = xq_pool.tile([P, ko, P], f8, tag="xq")
                    # x_q = fp8(x * (recip_x / 2)) on the vector engine
                    nc.vector.tensor_scalar_mul(xq[:], xf[:], rx_half)

                    psums = [
                        psum_pool.tile([P, oc], f32, space="PSUM",
                                       name="ps", tag="ps")
                        for i in range(cpp)
                    ]
                    for k2 in range(ko // 2):
                        lhsT = xq[:, 2 * k2:2 * k2 + 2, :]
                        for i in range(cpp):
                            nc.tensor.matmul(
                                psums[i][:],
                                lhsT,
                                wq_chunks[i][:, 2 * k2:2 * k2 + 2, :],
                                start=(k2 == 0),
                                stop=(k2 == ko // 2 - 1),
                                perf_mode=DR,
                            )

                    osb = out_pool.tile([P, cpp * oc], f32, tag="osb")
                    for i in range(cpp):
                        # out = psum * (4*sx*sw) + bias, fused on the vector engine
                        nc.vector.scalar_tensor_tensor(
                            osb[:, i * oc:(i + 1) * oc],
                            psums[i][:],
                            c4,
                            biasb_t[:, (p * cpp + i) * oc:(p * cpp + i + 1) * oc],
                            mybir.AluOpType.mult,
                            mybir.AluOpType.add,
                        )
                    nc.sync.dma_start(
                        out[r][:, p * cpp * oc:(p + 1) * cpp * oc], osb[:]
                    )

    nc.compile()
    return nc


_NC_CACHE = {}


def _get_nc(key=None):
    if key not in _NC_CACHE:
        _NC_CACHE[key] = _build_nc()
    return _NC_CACHE[key]


def _host_scales(inp, weight):
    """Replicate the reference's fp32 scale arithmetic exactly."""
    amax_w = np.max(np.abs(weight)).astype(np.float32)
    w_scale = amax_w / F8_MAX
    recip_w = np.float32(1.0) / w_scale

    amax_x = np.max(np.abs(inp)).astype(np.float32)
    x_scale = amax_x / F8_MAX
    recip_x = np.float32(1.0) / x_scale

    c4 = np.float32(4.0) * (x_scale * w_scale)
    rx_half = recip_x * np.float32(0.5)
    rw_half = recip_w * np.float32(0.5)
    return rx_half, rw_half, c4


def kernel(inp, weight, bias):
    return _run(inp, weight, bias)[0]


def _run(inp, weight, bias, trace=False, **kwargs):
    from concourse.bass_utils import run_bass_kernel_spmd

    inp = np.asarray(inp)
    weight = np.asarray(weight)
    bias = np.asarray(bias)

    rx_half, rw_half, c4 = _host_scales(inp, weight)
    consts = np.zeros((P, 4), np.float32)
    consts[:, 0] = rx_half
    consts[:, 1] = rw_half
    consts[:, 2] = c4

    x2 = inp.reshape(ROWS, K)

    # Pre-tile x row-shards: xt[r, ki, ko, col] = x_shard[r*128+col, ko*128+ki]
    xts = []
    for s in range(ROW_SHARDS):
        xs = x2[s * ROWS_C:(s + 1) * ROWS_C]
        xt = np.ascontiguousarray(
            xs.reshape(RT, P, KO, P).transpose(0, 3, 2, 1))
        xts.append(xt)

    # Pre-tile w col-shards: wt[c, ki, ko, col] = w_shard[ko*128+ki, c*512+col]
    wts, biasbs = [], []
    for s in range(COL_SHARDS):
        ws = weight[:, s * OUTF_C:(s + 1) * OUTF_C]
        wt = np.ascontiguousarray(
            ws.reshape(KO, P, NCHUNK, OC).transpose(2, 1, 0, 3))
        wts.append(wt)
        bs = bias[s * OUTF_C:(s + 1) * OUTF_C]
        biasbs.append(np.ascontiguousarray(
            np.broadcast_to(bs[None, :], (P, OUTF_C))))

    in_maps = []
    for c in range(N_CORES):
        rs, cs = divmod(c, COL_SHARDS)
        in_maps.append({
            "xt": xts[rs],
            "wt": wts[cs],
            "biasb": biasbs[cs],
            "consts": consts,
        })

    nc = _get_nc()
    res = run_bass_kernel_spmd(
        nc, in_maps, core_ids=list(range(N_CORES)), trace=trace, **kwargs
    )

    full = np.empty((ROWS, OUTF), np.float32)
    for c in range(N_CORES):
        rs, cs = divmod(c, COL_SHARDS)
        blk = res.results[c]["out"].reshape(ROWS_C, OUTF_C)
        full[rs * ROWS_C:(rs + 1) * ROWS_C, cs * OUTF_C:(cs + 1) * OUTF_C] = blk
    return full.reshape(B, T, OUTF), res


# revision 7
# speedup vs baseline: 1.1701x; 1.1701x over previous
"""FP8 dynamic-quantized linear (x @ W + b with abs-max fp8 quantization).

Strategy (8 NeuronCores):
  - Shard: 2-way on flattened batch*seq rows of inp, 4-way column-wise on
    weight out_features.  Each core computes a [4096, 4096] block of the
    [8192, 16384] output (K = 4096 contraction on-device).
  - The two scalar quantization scales (global abs-max of inp / weight) are
    computed on host and replicated to every core as tiny input tensors.
  - Everything else (fp8 quantization of x and W, fp8 DoubleRow matmul,
    dequant scale + bias epilogue) runs on-device.

fp8 format note: TRN float8e4 (= ml_dtypes.float8_e4m3, max 240, has inf)
differs from the reference's OCP float8_e4m3fn (max 448).  We quantize with
half the reference scale so post-scale values live in [-224, 224]; on the
power-of-2-relative e4m3 grid the RNE rounding then matches the reference's
e4m3fn rounding exactly (up to a negligible subnormal tail), and the factor
of 4 (2x per operand) is folded into the fp32 dequant scale.
"""

import numpy as np

F8_MAX = np.float32(448.0)

# ---- problem geometry (hardcoded per the task spec) ----
B, T, K, OUTF = 4, 2048, 4096, 16384
ROWS = B * T                     # 8192
N_CORES = 8
ROW_SHARDS, COL_SHARDS = 2, 4
ROWS_C = ROWS // ROW_SHARDS      # 4096 rows per core
OUTF_C = OUTF // COL_SHARDS      # 4096 out-features per core

P = 128                          # SBUF partitions
KO = K // P                      # 32 k-subtiles
RT = ROWS_C // P                 # 32 row tiles per core
OC = 512                         # out-feature chunk (psum free dim)
NCHUNK = OUTF_C // OC            # 8 chunks per core
NPASS = 2                        # chunk passes (4 chunks resident per pass)
CPP = NCHUNK // NPASS            # chunks per pass = 4
KH = 8                           # ko-slices per w staging DMA


def _build_nc(rt=RT, ko=KO, nchunk=NCHUNK, npass=NPASS, oc=OC):
    """Build the per-core SPMD bass program (same program on all 8 cores)."""
    import concourse.bass as bass
    import concourse.tile as tile
    from concourse import bacc, mybir

    cpp = nchunk // npass
    outf_c = nchunk * oc
    f32 = mybir.dt.float32
    f8 = mybir.dt.float8e4
    DR = mybir.MatmulPerfMode.DoubleRow

    nc = bacc.Bacc(
        "TRN2",
        target_bir_lowering=False,
        debug=False,
        enable_asserts=False,
        num_devices=N_CORES,
    )

    xt = nc.dram_tensor("xt", [rt, P, ko, P], f32, kind="ExternalInput").ap()
    wt = nc.dram_tensor("wt", [nchunk, P, ko, oc], f32, kind="ExternalInput").ap()
    biasb = nc.dram_tensor("biasb", [P, outf_c], f32, kind="ExternalInput").ap()
    consts = nc.dram_tensor("consts", [P, 4], f32, kind="ExternalInput").ap()
    out = nc.dram_tensor("out", [rt, P, outf_c], f32, kind="ExternalOutput").ap()

    kh = min(KH, ko)

    with tile.TileContext(nc) as tc:
        # DMA queue split: x loads ride the SP (sync) HWDGE FIFO; w loads and
        # out stores ride the ACT (scalar) HWDGE FIFO.  With a single FIFO the
        # next row's x load queues behind the previous row's out store (which
        # waits on its eviction), stalling the PE ~4.4us per row tile.
        with (
            tc.tile_pool(name="const", bufs=1) as const_pool,
            tc.tile_pool(name="wq", bufs=min(nchunk, cpp + 2)) as wq_pool,
            tc.tile_pool(name="xq", bufs=3) as xq_pool,
            tc.tile_pool(name="xf", bufs=2) as xf_pool,
            tc.tile_pool(name="wf", bufs=2) as wf_pool,
            tc.tile_pool(name="osb", bufs=2) as out_pool,
            tc.tile_pool(name="psum", bufs=8, space="PSUM") as psum_pool,
        ):
            consts_t = const_pool.tile([P, 4], f32)
            nc.sync.dma_start(consts_t[:], consts)
            rx_half = consts_t[:, 0:1]
            rw_half = consts_t[:, 1:2]
            c4 = consts_t[:, 2:3]

            biasb_t = const_pool.tile([P, outf_c], f32)
            nc.scalar.dma_start(biasb_t[:], biasb)

            def load_wq_chunk(c):
                wq_c = wq_pool.tile([P, ko, oc], f8, tag="wq")
                for h in range(ko // kh):
                    wf = wf_pool.tile([P, kh, oc], f32, tag="wf")
                    nc.scalar.dma_start(wf[:], wt[c, :, h * kh:(h + 1) * kh, :])
                    # w_q = fp8(w * (recip_w / 2)) on the scalar engine
                    nc.scalar.mul(wq_c[:, h * kh:(h + 1) * kh, :], wf[:], rw_half)
                return wq_c

            for p in range(npass):
                if p == 0:
                    wq_chunks = [load_wq_chunk(i) for i in range(cpp)]
                else:
                    wq_chunks = next_chunks
                next_chunks = []
                for r in range(rt):
                    # stagger next-pass chunk prefetches into the middle of
                    # this pass (extra wq slots are free by then)
                    if p + 1 < npass and r in (rt // 4, rt // 2) and len(next_chunks) < cpp:
                        next_chunks.append(load_wq_chunk((p + 1) * cpp + len(next_chunks)))
                    xf = xf_pool.tile([P, ko, P], f32, tag="xf")
                    nc.sync.dma_start(xf[:], xt[r])
                    xq = xq_pool.tile([P, ko, P], f8, tag="xq")
                    # x_q = fp8(x * (recip_x / 2)) on the vector engine
                    nc.vector.tensor_scalar_mul(xq[:], xf[:], rx_half)

                    psums = [
                        psum_pool.tile([P, oc], f32, space="PSUM",
                                       name="ps", tag="ps")
                        for i in range(cpp)
                    ]
                    for k2 in range(ko // 2):
                        lhsT = xq[:, 2 * k2:2 * k2 + 2, :]
                        for i in range(cpp):
                            nc.tensor.matmul(
                                psums[i][:],
                                lhsT,
                                wq_chunks[i][:, 2 * k2:2 * k2 + 2, :],
                                start=(k2 == 0),
                                stop=(k2 == ko // 2 - 1),
                                perf_mode=DR,
                            )

                    osb = out_pool.tile([P, cpp * oc], f32, tag="osb")
                    for i in range(cpp):
                        # out = psum * (4*sx*sw) + bias, fused on the vector engine
                        nc.vector.scalar_tensor_tensor(
                            osb[:, i * oc:(i + 1) * oc],
                            psums[i][:],
                            c4,
                            biasb_t[:, (p * cpp + i) * oc:(p * cpp + i + 1) * oc],
                            mybir.AluOpType.mult,
                            mybir.AluOpType.add,
                        )
                    nc.scalar.dma_start(
                        out[r][:, p * cpp * oc:(p + 1) * cpp * oc], osb[:]
                    )
                if p + 1 < npass:
                    while len(next_chunks) < cpp:
                        next_chunks.append(
                            load_wq_chunk((p + 1) * cpp + len(next_chunks)))

    nc.compile()
    return nc


_NC_CACHE = {}


def _get_nc(key=None):
    if key not in _NC_CACHE:
        _NC_CACHE[key] = _build_nc()
    return _NC_CACHE[key]


def _host_scales(inp, weight):
    """Replicate the reference's fp32 scale arithmetic exactly."""
    amax_w = np.max(np.abs(weight)).astype(np.float32)
    w_scale = amax_w / F8_MAX
    recip_w = np.float32(1.0) / w_scale

    amax_x = np.max(np.abs(inp)).astype(np.float32)
    x_scale = amax_x / F8_MAX
    recip_x = np.float32(1.0) / x_scale

    c4 = np.float32(4.0) * (x_scale * w_scale)
    rx_half = recip_x * np.float32(0.5)
    rw_half = recip_w * np.float32(0.5)
    return rx_half, rw_half, c4


def kernel(inp, weight, bias):
    return _run(inp, weight, bias)[0]


def _run(inp, weight, bias, trace=False, **kwargs):
    from concourse.bass_utils import run_bass_kernel_spmd

    inp = np.asarray(inp)
    weight = np.asarray(weight)
    bias = np.asarray(bias)

    rx_half, rw_half, c4 = _host_scales(inp, weight)
    consts = np.zeros((P, 4), np.float32)
    consts[:, 0] = rx_half
    consts[:, 1] = rw_half
    consts[:, 2] = c4

    x2 = inp.reshape(ROWS, K)

    # Pre-tile x row-shards: xt[r, ki, ko, col] = x_shard[r*128+col, ko*128+ki]
    xts = []
    for s in range(ROW_SHARDS):
        xs = x2[s * ROWS_C:(s + 1) * ROWS_C]
        xt = np.ascontiguousarray(
            xs.reshape(RT, P, KO, P).transpose(0, 3, 2, 1))
        xts.append(xt)

    # Pre-tile w col-shards: wt[c, ki, ko, col] = w_shard[ko*128+ki, c*512+col]
    wts, biasbs = [], []
    for s in range(COL_SHARDS):
        ws = weight[:, s * OUTF_C:(s + 1) * OUTF_C]
        wt = np.ascontiguousarray(
            ws.reshape(KO, P, NCHUNK, OC).transpose(2, 1, 0, 3))
        wts.append(wt)
        bs = bias[s * OUTF_C:(s + 1) * OUTF_C]
        biasbs.append(np.ascontiguousarray(
            np.broadcast_to(bs[None, :], (P, OUTF_C))))

    in_maps = []
    for c in range(N_CORES):
        rs, cs = divmod(c, COL_SHARDS)
        in_maps.append({
            "xt": xts[rs],
            "wt": wts[cs],
            "biasb": biasbs[cs],
            "consts": consts,
        })

    nc = _get_nc()
    res = run_bass_kernel_spmd(
        nc, in_maps, core_ids=list(range(N_CORES)), trace=trace, **kwargs
    )

    full = np.empty((ROWS, OUTF), np.float32)
    for c in range(N_CORES):
        rs, cs = divmod(c, COL_SHARDS)
        blk = res.results[c]["out"].reshape(ROWS_C, OUTF_C)
        full[rs * ROWS_C:(rs + 1) * ROWS_C, cs * OUTF_C:(cs + 1) * OUTF_C] = blk
    return full.reshape(B, T, OUTF), res


# revision 10
# speedup vs baseline: 1.2284x; 1.0499x over previous
"""FP8 dynamic-quantized linear (x @ W + b with abs-max fp8 quantization).

Strategy (8 NeuronCores):
  - Shard: 2-way on flattened batch*seq rows of inp, 4-way column-wise on
    weight out_features.  Each core computes a [4096, 4096] block of the
    [8192, 16384] output (K = 4096 contraction on-device).
  - The two scalar quantization scales (global abs-max of inp / weight) are
    computed on host and replicated to every core as tiny input tensors.
  - Everything else (fp8 quantization of x and W, fp8 DoubleRow matmul,
    dequant scale + bias epilogue) runs on-device.

fp8 format note: TRN float8e4 (= ml_dtypes.float8_e4m3, max 240, has inf)
differs from the reference's OCP float8_e4m3fn (max 448).  We quantize with
half the reference scale so post-scale values live in [-224, 224]; on the
power-of-2-relative e4m3 grid the RNE rounding then matches the reference's
e4m3fn rounding exactly (up to a negligible subnormal tail), and the factor
of 4 (2x per operand) is folded into the fp32 dequant scale.
"""

import numpy as np

F8_MAX = np.float32(448.0)

# ---- problem geometry (hardcoded per the task spec) ----
B, T, K, OUTF = 4, 2048, 4096, 16384
ROWS = B * T                     # 8192
N_CORES = 8
ROW_SHARDS, COL_SHARDS = 1, 8
ROWS_C = ROWS // ROW_SHARDS      # 8192 rows per core (replicated x)
OUTF_C = OUTF // COL_SHARDS      # 2048 out-features per core

P = 128                          # SBUF partitions
KO = K // P                      # 32 k-subtiles
RT = ROWS_C // P                 # 64 row tiles per core
OC = 512                         # out-feature chunk (psum free dim)
NCHUNK = OUTF_C // OC            # 4 chunks per core
NPASS = 1                        # all 4 chunks resident, single pass
CPP = NCHUNK // NPASS            # chunks per pass = 4
KH = 8                           # ko-slices per w staging DMA


def _build_nc(rt=RT, ko=KO, nchunk=NCHUNK, npass=NPASS, oc=OC):
    """Build the per-core SPMD bass program (same program on all 8 cores)."""
    import concourse.bass as bass
    import concourse.tile as tile
    from concourse import bacc, mybir

    cpp = nchunk // npass
    outf_c = nchunk * oc
    f32 = mybir.dt.float32
    f8 = mybir.dt.float8e4
    DR = mybir.MatmulPerfMode.DoubleRow

    nc = bacc.Bacc(
        "TRN2",
        target_bir_lowering=False,
        debug=False,
        enable_asserts=False,
        num_devices=N_CORES,
    )

    xt = nc.dram_tensor("xt", [rt, P, ko, P], f32, kind="ExternalInput").ap()
    wt = nc.dram_tensor("wt", [nchunk, P, ko, oc], f32, kind="ExternalInput").ap()
    biasb = nc.dram_tensor("biasb", [P, outf_c], f32, kind="ExternalInput").ap()
    consts = nc.dram_tensor("consts", [P, 4], f32, kind="ExternalInput").ap()
    out = nc.dram_tensor("out", [rt, P, outf_c], f32, kind="ExternalOutput").ap()

    kh = min(KH, ko)

    with tile.TileContext(nc) as tc:
        # DMA queue split: x loads ride the SP (sync) HWDGE FIFO; w loads and
        # out stores ride the ACT (scalar) HWDGE FIFO.  With a single FIFO the
        # next row's x load queues behind the previous row's out store (which
        # waits on its eviction), stalling the PE ~4.4us per row tile.
        with (
            tc.tile_pool(name="const", bufs=1) as const_pool,
            tc.tile_pool(name="wq", bufs=min(nchunk, cpp + 3)) as wq_pool,
            tc.tile_pool(name="xq", bufs=3) as xq_pool,
            tc.tile_pool(name="xf", bufs=2) as xf_pool,
            tc.tile_pool(name="wf", bufs=2) as wf_pool,
            tc.tile_pool(name="osb", bufs=2) as out_pool,
            tc.tile_pool(name="psum", bufs=8, space="PSUM") as psum_pool,
        ):
            consts_t = const_pool.tile([P, 4], f32)
            nc.sync.dma_start(consts_t[:], consts)
            rx_half = consts_t[:, 0:1]
            rw_half = consts_t[:, 1:2]
            c4 = consts_t[:, 2:3]

            biasb_t = const_pool.tile([P, outf_c], f32)
            nc.scalar.dma_start(biasb_t[:], biasb)

            def load_wq_slice(wq_c, c, h):
                wf = wf_pool.tile([P, kh, oc], f32, tag="wf")
                nc.scalar.dma_start(wf[:], wt[c, :, h * kh:(h + 1) * kh, :])
                # w_q = fp8(w * (recip_w / 2)) on the scalar engine
                nc.scalar.mul(wq_c[:, h * kh:(h + 1) * kh, :], wf[:], rw_half)

            def load_wq_chunk(c):
                wq_c = wq_pool.tile([P, ko, oc], f8, tag="wq")
                for h in range(ko // kh):
                    load_wq_slice(wq_c, c, h)
                return wq_c

            prefetch_at = [rt // 5, 2 * rt // 5, 3 * rt // 5]

            for p in range(npass):
                if p == 0:
                    # kh-major interleaved load: the first k-slices of all
                    # chunks land first, so the PE's k2 ladder can start
                    # after ~1/4 of the w bytes instead of all of them.
                    wq_chunks = [wq_pool.tile([P, ko, oc], f8, tag="wq",
                                              name="wq") for _ in range(cpp)]
                    for h in range(ko // kh):
                        for i, wq_c in enumerate(wq_chunks):
                            load_wq_slice(wq_c, i, h)
                else:
                    wq_chunks = next_chunks
                next_chunks = []
                for r in range(rt):
                    # stagger next-pass chunk prefetches into this pass
                    # (extra wq slots are free by then)
                    if p + 1 < npass and r in prefetch_at and len(next_chunks) < cpp:
                        next_chunks.append(load_wq_chunk((p + 1) * cpp + len(next_chunks)))
                    xf = xf_pool.tile([P, ko, P], f32, tag="xf")
                    nc.sync.dma_start(xf[:], xt[r])
                    xq = xq_pool.tile([P, ko, P], f8, tag="xq")
                    # x_q = fp8(x * (recip_x / 2)) on the vector engine
                    nc.vector.tensor_scalar_mul(xq[:], xf[:], rx_half)

                    psums = [
                        psum_pool.tile([P, oc], f32, space="PSUM",
                                       name="ps", tag="ps")
                        for i in range(cpp)
                    ]
                    for k2 in range(ko // 2):
                        lhsT = xq[:, 2 * k2:2 * k2 + 2, :]
                        for i in range(cpp):
                            nc.tensor.matmul(
                                psums[i][:],
                                lhsT,
                                wq_chunks[i][:, 2 * k2:2 * k2 + 2, :],
                                start=(k2 == 0),
                                stop=(k2 == ko // 2 - 1),
                                perf_mode=DR,
                            )

                    osb = out_pool.tile([P, cpp * oc], f32, tag="osb")
                    for i in range(cpp):
                        # out = psum * (4*sx*sw) + bias, fused on the vector engine
                        nc.vector.scalar_tensor_tensor(
                            osb[:, i * oc:(i + 1) * oc],
                            psums[i][:],
                            c4,
                            biasb_t[:, (p * cpp + i) * oc:(p * cpp + i + 1) * oc],
                            mybir.AluOpType.mult,
                            mybir.AluOpType.add,
                        )
                    nc.scalar.dma_start(
                        out[r][:, p * cpp * oc:(p + 1) * cpp * oc], osb[:]
                    )
                if p + 1 < npass:
                    while len(next_chunks) < cpp:
                        next_chunks.append(
                            load_wq_chunk((p + 1) * cpp + len(next_chunks)))

    nc.compile()
    return nc


_NC_CACHE = {}


def _get_nc(key=None):
    if key not in _NC_CACHE:
        _NC_CACHE[key] = _build_nc()
    return _NC_CACHE[key]


def _host_scales(inp, weight):
    """Replicate the reference's fp32 scale arithmetic exactly."""
    amax_w = np.max(np.abs(weight)).astype(np.float32)
    w_scale = amax_w / F8_MAX
    recip_w = np.float32(1.0) / w_scale

    amax_x = np.max(np.abs(inp)).astype(np.float32)
    x_scale = amax_x / F8_MAX
    recip_x = np.float32(1.0) / x_scale

    c4 = np.float32(4.0) * (x_scale * w_scale)
    rx_half = recip_x * np.float32(0.5)
    rw_half = recip_w * np.float32(0.5)
    return rx_half, rw_half, c4


def kernel(inp, weight, bias):
    return _run(inp, weight, bias)[0]


def _run(inp, weight, bias, trace=False, **kwargs):
    from concourse.bass_utils import run_bass_kernel_spmd

    inp = np.asarray(inp)
    weight = np.asarray(weight)
    bias = np.asarray(bias)

    rx_half, rw_half, c4 = _host_scales(inp, weight)
    consts = np.zeros((P, 4), np.float32)
    consts[:, 0] = rx_half
    consts[:, 1] = rw_half
    consts[:, 2] = c4

    x2 = inp.reshape(ROWS, K)

    # Pre-tile x row-shards: xt[r, ki, ko, col] = x_shard[r*128+col, ko*128+ki]
    xts = []
    for s in range(ROW_SHARDS):
        xs = x2[s * ROWS_C:(s + 1) * ROWS_C]
        xt = np.ascontiguousarray(
            xs.reshape(RT, P, KO, P).transpose(0, 3, 2, 1))
        xts.append(xt)

    # Pre-tile w col-shards: wt[c, ki, ko, col] = w_shard[ko*128+ki, c*512+col]
    wts, biasbs = [], []
    for s in range(COL_SHARDS):
        ws = weight[:, s * OUTF_C:(s + 1) * OUTF_C]
        wt = np.ascontiguousarray(
            ws.reshape(KO, P, NCHUNK, OC).transpose(2, 1, 0, 3))
        wts.append(wt)
        bs = bias[s * OUTF_C:(s + 1) * OUTF_C]
        biasbs.append(np.ascontiguousarray(
            np.broadcast_to(bs[None, :], (P, OUTF_C))))

    in_maps = []
    for c in range(N_CORES):
        rs, cs = divmod(c, COL_SHARDS)
        in_maps.append({
            "xt": xts[rs],
            "wt": wts[cs],
            "biasb": biasbs[cs],
            "consts": consts,
        })

    nc = _get_nc()
    res = run_bass_kernel_spmd(
        nc, in_maps, core_ids=list(range(N_CORES)), trace=trace, **kwargs
    )

    full = np.empty((ROWS, OUTF), np.float32)
    for c in range(N_CORES):
        rs, cs = divmod(c, COL_SHARDS)
        blk = res.results[c]["out"].reshape(ROWS_C, OUTF_C)
        full[rs * ROWS_C:(rs + 1) * ROWS_C, cs * OUTF_C:(cs + 1) * OUTF_C] = blk
    return full.reshape(B, T, OUTF), res
